# revision 31
# baseline (speedup 1.0000x reference)
"""AdaptiveTokenMixer Trainium2 kernel (8 NeuronCores, pure data parallel).

Per-core algorithm (one batch element per core), pipelined over 2 chunks
(18+17 position-blocks of BLK=120 outputs) mapped to the two HWDGE rings:
  1. alpha stage: delta_times/valid_mask host-packed into one [70, 136]
     row tensor; two PE transposes per tap produce both windows; t1 =
     BIG - dt fused into the scalar-engine PSUM evictions; cv/lg computed
     per-strip under the transpose stream; masked temporal-decay softmax
     over K=8 offsets (elementwise split across vector/gpsimd), blended as
     au = (e + s*c)*cv (scale-invariant rewrite avoids the reciprocal);
     alpha finalized per chunk -> af bf16.
  2. W stage (per chunk): af chunk written to a DRAM scratch with a SKEWED
     access pattern (banded W^T[m, k] = alpha[n0+m, k-m], m-major 128x128
     tiles over a zeros-initialized buffer); loaded back with an
     XBAR-transposing DMA into W[k, m] orientation. The skew and its XBAR
     load MUST be issued on opposite HWDGE rings (SP=sync / Act=scalar):
     a same-ring consumer's semaphore wait is elided under the ring-FIFO
     assumption, but the XBAR read races the skew's multi-engine
     descriptor drain (observed nondeterministic corruption).
  3. Mix (per block): out[m, :] = sum_k W[k, m] * x[n0+k, :] -- one
     128x128 @ 128x256 bf16 matmul per block (PSUM f32), two blocks per
     PSUM bank.
  4. Evict PSUM -> SBUF bf16 (paired, alternating vector/scalar), 5
     group-stores overlapping later matmuls.

Self-contained: hardcodes shapes for B=8, N=4096, d=256, K=8.
"""
import numpy as np
import ml_dtypes

import concourse.bass as bass
import concourse.bacc as bacc
import concourse.mybir as mybir
from concourse import tile
from concourse.bass_utils import run_bass_kernel_spmd

B, N, D, K = 8, 4096, 256, 8
BLK = 120                      # output positions per block
NB = (N + BLK - 1) // BLK      # 35 blocks -> covers 4200 positions
NOUT = NB * BLK                # 4200 rows in padded device output
NPAD = 4224                    # padded input length (>= 34*120 + 136)
KW = 128                       # k-window (contraction) per block
WBLK = KW * KW                 # W scratch elements per block
F = K * NB                     # alpha free size (b-major, p-minor)
BIG = 1024.0
CA = 18                        # chunk A blocks (skew on SP, load on Act)
CB = NB - CA                   # chunk B blocks (skew on Act, load on SP)
G = 5                          # store groups
GB = NB // G                   # blocks per store group (7)

_CACHE = {}


def _build():
    nc = bacc.Bacc("TRN2", target_bir_lowering=False, debug=False,
                   num_devices=B)
    f32 = mybir.dt.float32
    bf16 = mybir.dt.bfloat16

    x_t = nc.dram_tensor("x", [NPAD, D], bf16, kind="ExternalInput")
    dvf_t = nc.dram_tensor("dvf", [99, 136], f32, kind="ExternalInput")
    vfr_t = nc.dram_tensor("vfr", [35, 136], f32, kind="ExternalInput")
    bwsm_t = nc.dram_tensor("bwsm", [128, K], f32, kind="ExternalInput")
    idf_t = nc.dram_tensor("idf", [128, 128], f32, kind="ExternalInput")
    wza_t = nc.dram_tensor("wza", [CA * WBLK], bf16, kind="ExternalInput")
    wzb_t = nc.dram_tensor("wzb", [CB * WBLK], bf16, kind="ExternalInput")
    out_t = nc.dram_tensor("out", [NOUT, D], bf16, kind="ExternalOutput")

    def pb(t):  # [128,(b,p)] view -> [128, b, p] (p innermost, for reduces)
        return bass.AP(t.tensor, t.offset, [t.ap[0], [K, NB], [1, K]])

    def strip(t, p):  # tap-p strip [128, NB] (stride K)
        return bass.AP(t.tensor, t.offset + p, [t.ap[0], [K, NB]])

    def exp_nb_h(a, b0, nb):  # block-range slice of an exp_nb broadcast
        return bass.AP(a.tensor, a.offset + b0 * a.ap[1][0],
                       [a.ap[0], [a.ap[1][0], nb], [0, K]])

    def exp_k_h(a, nb):       # block-range slice of an exp_k broadcast
        return bass.AP(a.tensor, a.offset, [a.ap[0], [0, nb], [1, K]])

    NB1 = 24                  # block split for vector/gpsimd halves
    NB2 = NB - NB1
    F1 = NB1 * K

    def tt2(out, a, b2, op, b2h=None, ah=None):
        # tensor_tensor split across vector/gpsimd at block boundary NB1.
        if b2h is None:
            b2h = (b2[:, :F1], b2[:, F1:])
        if ah is None:
            ah = (a[:, :F1], a[:, F1:])
        nc.vector.tensor_tensor(out[:, :F1], ah[0], b2h[0], op)
        nc.gpsimd.tensor_tensor(out[:, F1:], ah[1], b2h[1], op)

    with tile.TileContext(nc) as tc:
        with tc.tile_pool(name="alph", bufs=1) as apool, \
             tc.tile_pool(name="outg", bufs=3) as opool, \
             tc.tile_pool(name="big", bufs=1) as bpool, \
             tc.tile_pool(name="psA", bufs=2, space="PSUM") as psA, \
             tc.tile_pool(name="psB", bufs=4, space="PSUM") as psB:

            # ---- input / constant loads (sync = SP ring) ----
            dvf = apool.tile([99, 136], f32)
            nc.sync.dma_start(dvf[:], dvf_t.ap())
            ident_f = apool.tile([128, 128], f32)
            nc.sync.dma_start(ident_f[:], idf_t.ap())
            vfr = apool.tile([35, 136], f32)
            nc.sync.dma_start(vfr[:], vfr_t.ap())
            bwsm = apool.tile([128, K], f32)
            nc.sync.dma_start(bwsm[:], bwsm_t.ap())
            # x windows in two chunks: x_all[i, b, d] = x[b*120+i, d]
            x_all = bpool.tile([128, NB, D], bf16)
            for c0, cn in ((0, CA), (CA, CB)):
                nc.sync.dma_start(
                    x_all[:, c0:c0 + cn, :],
                    bass.AP(x_t, c0 * BLK * D,
                            [[D, 128], [BLK * D, cn], [1, D]]))

            # ---- window strips: two PE transposes per tap ----
            # dvf rows 0..34 = dt blocks, rows 64..98 = vf blocks.
            # t1 = BIG - dt fused into the scalar PSUM eviction.
            t1 = apool.tile([128, F], f32)
            vw = apool.tile([128, F], f32)
            cv = apool.tile([128, F], f32)
            lg = apool.tile([128, F], f32)
            for p in range(K):
                ptw = psA.tile([128, 70], f32, tag="win")
                nc.tensor.transpose(ptw[:, 0:NB], dvf[:NB, p:p + 128],
                                    ident_f[:NB, :NB])
                nc.tensor.transpose(ptw[:, NB:70], vfr[:NB, p:p + 128],
                                    ident_f[:NB, :NB])
                nc.scalar.activation(strip(t1, p), ptw[:, 0:NB],
                                     mybir.ActivationFunctionType.Copy,
                                     bias=BIG, scale=-1.0)
                nc.vector.tensor_copy(strip(vw, p), ptw[:, NB:70])
                # cv_p = vw_p * vw_0; lg_p = t1_p * cv_p (under the stream)
                nc.vector.tensor_tensor(strip(cv, p), strip(vw, p),
                                        strip(vw, 0), mybir.AluOpType.mult)
                nc.gpsimd.tensor_tensor(strip(lg, p), strip(t1, p),
                                        strip(cv, p), mybir.AluOpType.mult)

            # ---- alpha stage, fully per chunk ----
            # chunk A: elementwise on vector; chunk B: elementwise on
            # gpsimd (free-axis reduces are vector-only for both). Chunk A
            # finishes first; its skew is emitted immediately below.
            mx = apool.tile([128, NB], f32)
            s = apool.tile([128, NB], f32)
            sa = apool.tile([128, NB], f32)
            r = apool.tile([128, NB], f32)
            ei = apool.tile([128, F], f32)
            e = apool.tile([128, F], f32)
            sc = apool.tile([128, F], f32)
            au = apool.tile([128, F], f32)
            af = apool.tile([128, F], bf16)

            def pb_h(t, c0, cn):
                return bass.AP(t.tensor, t.offset + c0 * K,
                               [t.ap[0], [K, cn], [1, K]])

            def strip_h(t, p, c0, cn):
                return bass.AP(t.tensor, t.offset + p + c0 * K,
                               [t.ap[0], [K, cn]])

            for c0, cn, eng, wt in ((0, CA, nc.vector, wza_t),
                                    (CA, CB, nc.gpsimd, wzb_t)):
                f0, f1 = c0 * K, (c0 + cn) * K
                nc.vector.tensor_reduce(mx[:, c0:c0 + cn], pb_h(lg, c0, cn),
                                        mybir.AxisListType.X,
                                        mybir.AluOpType.max)
                eng.tensor_tensor(ei[:, f0:f1], lg[:, f0:f1],
                                  exp_nb_h(mx[:, :], c0, cn),
                                  mybir.AluOpType.subtract)
                nc.scalar.activation(e[:, f0:f1], ei[:, f0:f1],
                                     mybir.ActivationFunctionType.Exp)
                nc.vector.tensor_reduce(s[:, c0:c0 + cn], pb_h(e, c0, cn),
                                        mybir.AxisListType.X,
                                        mybir.AluOpType.add)
                # scale-invariant blend: au = (e + s*c) * cv (c = bwsm row)
                eng.tensor_tensor(sc[:, f0:f1], exp_nb_h(s[:, :], c0, cn),
                                  exp_k_h(bwsm[:, :], cn),
                                  mybir.AluOpType.mult)
                eng.tensor_tensor(au[:, f0:f1], sc[:, f0:f1], e[:, f0:f1],
                                  mybir.AluOpType.add)
                eng.tensor_tensor(au[:, f0:f1], au[:, f0:f1], cv[:, f0:f1],
                                  mybir.AluOpType.mult)
                nc.vector.tensor_reduce(sa[:, c0:c0 + cn], pb_h(au, c0, cn),
                                        mybir.AxisListType.X,
                                        mybir.AluOpType.add)
                nc.vector.tensor_scalar(sa[:, c0:c0 + cn], sa[:, c0:c0 + cn],
                                        1e-8, None, mybir.AluOpType.max)
                nc.vector.reciprocal(r[:, c0:c0 + cn], sa[:, c0:c0 + cn])
                nc.vector.tensor_tensor(r[:, c0:c0 + cn], r[:, c0:c0 + cn],
                                        strip_h(vw, 0, c0, cn),
                                        mybir.AluOpType.mult)
                eng.tensor_tensor(af[:, f0:f1], au[:, f0:f1],
                                  exp_nb_h(r[:, :], c0, cn),
                                  mybir.AluOpType.mult)
                # skewed W write for this chunk (SP ring):
                # W^T[b][m, m+p] = af[m, p, b]
                nc.sync.dma_start(
                    bass.AP(wt, 0, [[KW + 1, BLK], [WBLK, cn], [1, K]]),
                    bass.AP(af.tensor, af.offset + c0 * K,
                            [af.ap[0], [K, cn], [1, K]])[:BLK, :, :])

            # NOTE: the XBAR is one shared unit -- concurrent DMA_TRANSPOSE
            # instructions from the two HWDGE rings corrupt each other.
            # All transposes must serialize on one ring (Act).
            w_all = bpool.tile([128, NB, KW], bf16)
            nc.scalar.dma_start(
                w_all[:, 0:CA, :],
                bass.AP(wza_t, 0, [[KW, CA * KW], [1, KW]]),
                transpose=True)
            nc.scalar.dma_start(
                w_all[:, CA:NB, :],
                bass.AP(wzb_t, 0, [[KW, CB * KW], [1, KW]]),
                transpose=True)

            # ---- per-block banded matmul; paired evict; group stores ----
            for g in range(G):
                out_g = opool.tile([128, GB, D], bf16, tag="og")
                for j in range(0, GB, 2):
                    b = g * GB + j
                    npair = min(2, GB - j)
                    pt = psB.tile([KW, 2 * D], f32, tag="mm")
                    for q in range(npair):
                        nc.tensor.matmul(pt[:, q * D:(q + 1) * D],
                                         w_all[:, b + q, :],
                                         x_all[:, b + q, :])
                    if (j // 2) % 2 == 1:
                        nc.scalar.copy(out_g[:BLK, j:j + npair, :],
                                       pt[:BLK, :npair * D])
                    else:
                        nc.vector.tensor_copy(out_g[:BLK, j:j + npair, :],
                                              pt[:BLK, :npair * D])
                nc.sync.dma_start(
                    bass.AP(out_t, g * GB * BLK * D,
                            [[D, BLK], [BLK * D, GB], [1, D]]),
                    out_g[:BLK, :, :])
    nc.compile()
    return nc


def _get_nc():
    if "nc" not in _CACHE:
        _CACHE["nc"] = _build()
    return _CACHE["nc"]


def _make_in_maps(x, delta_times, valid_mask, w, beta):
    w64 = w.astype(np.float64)
    wsm = np.exp(w64 - w64.max())
    wsm /= wsm.sum()
    b = 1.0 / (1.0 + np.exp(-float(beta[0])))
    bwsm = np.tile((b / (1.0 - b) * wsm)[None, :], (128, 1)).astype(np.float32)
    ident = np.eye(128, dtype=np.float32)
    wza = np.zeros(CA * WBLK, np.float32).astype(ml_dtypes.bfloat16)
    wzb = np.zeros(CB * WBLK, np.float32).astype(ml_dtypes.bfloat16)

    in_maps = []
    for i in range(B):
        xp = np.zeros((NPAD, D), np.float32)
        xp[:N] = x[i]
        dtp = np.zeros(NPAD, np.float32)
        dtp[:N] = delta_times[i]
        vfp = np.zeros(NPAD, np.float32)
        vfp[:N] = valid_mask[i].astype(np.float32)
        dvf = np.zeros((99, 136), np.float32)
        for bb in range(NB):
            dvf[bb, :] = dtp[bb * BLK:bb * BLK + 136]
            dvf[64 + bb, :] = vfp[bb * BLK:bb * BLK + 136]
        in_maps.append({
            "x": xp.astype(ml_dtypes.bfloat16),
            "dvf": dvf,
            "vfr": dvf[64:99].copy(),
            "bwsm": bwsm,
            "idf": ident,
            "wza": wza,
            "wzb": wzb,
        })
    return in_maps


def _execute(in_maps, trace=False, **kw):
    nc = _get_nc()
    return run_bass_kernel_spmd(nc, in_maps, core_ids=list(range(B)),
                                trace=trace, **kw)


def kernel(x, delta_times, valid_mask, w, beta):
    in_maps = _make_in_maps(x, delta_times, valid_mask, w, beta)
    kr = _execute(in_maps, trace=False)
    outs = [kr.results[i]["out"][:N].astype(np.float32) for i in range(B)]
    return np.stack(outs, axis=0)


# revision 32
# speedup vs baseline: 1.0317x; 1.0317x over previous
"""AdaptiveTokenMixer Trainium2 kernel (8 NeuronCores, pure data parallel).

Per-core algorithm (one batch element per core), pipelined over 2 chunks
(18+17 position-blocks of BLK=120 outputs) mapped to the two HWDGE rings:
  1. alpha stage: delta_times/valid_mask host-packed into one [70, 136]
     row tensor; two PE transposes per tap produce both windows; t1 =
     BIG - dt fused into the scalar-engine PSUM evictions; cv/lg computed
     per-strip under the transpose stream; masked temporal-decay softmax
     over K=8 offsets (elementwise split across vector/gpsimd), blended as
     au = (e + s*c)*cv (scale-invariant rewrite avoids the reciprocal);
     alpha finalized per chunk -> af bf16.
  2. W stage (per chunk): af chunk written to a DRAM scratch with a SKEWED
     access pattern (banded W^T[m, k] = alpha[n0+m, k-m], m-major 128x128
     tiles over a zeros-initialized buffer); loaded back with an
     XBAR-transposing DMA into W[k, m] orientation. The skew and its XBAR
     load MUST be issued on opposite HWDGE rings (SP=sync / Act=scalar):
     a same-ring consumer's semaphore wait is elided under the ring-FIFO
     assumption, but the XBAR read races the skew's multi-engine
     descriptor drain (observed nondeterministic corruption).
  3. Mix (per block): out[m, :] = sum_k W[k, m] * x[n0+k, :] -- one
     128x128 @ 128x256 bf16 matmul per block (PSUM f32), two blocks per
     PSUM bank.
  4. Evict PSUM -> SBUF bf16 (paired, alternating vector/scalar), 5
     group-stores overlapping later matmuls.

Self-contained: hardcodes shapes for B=8, N=4096, d=256, K=8.
"""
import numpy as np
import ml_dtypes

import concourse.bass as bass
import concourse.bacc as bacc
import concourse.mybir as mybir
from concourse import tile
from concourse.bass_utils import run_bass_kernel_spmd

B, N, D, K = 8, 4096, 256, 8
BLK = 120                      # output positions per block
NB = (N + BLK - 1) // BLK      # 35 blocks -> covers 4200 positions
NOUT = NB * BLK                # 4200 rows in padded device output
NPAD = 4224                    # padded input length (>= 34*120 + 136)
KW = 128                       # k-window (contraction) per block
WBLK = KW * KW                 # W scratch elements per block
F = K * NB                     # alpha free size (b-major, p-minor)
BIG = 1024.0
CA = 18                        # chunk A blocks (skew on SP, load on Act)
CB = NB - CA                   # chunk B blocks (skew on Act, load on SP)
G = 5                          # store groups
GB = NB // G                   # blocks per store group (7)

_CACHE = {}


def _build():
    nc = bacc.Bacc("TRN2", target_bir_lowering=False, debug=False,
                   num_devices=B)
    f32 = mybir.dt.float32
    bf16 = mybir.dt.bfloat16

    x_t = nc.dram_tensor("x", [NPAD, D], bf16, kind="ExternalInput")
    cst_t = nc.dram_tensor("cst", [128, 272], f32, kind="ExternalInput")
    vfr_t = nc.dram_tensor("vfr", [35, 136], f32, kind="ExternalInput")
    wza_t = nc.dram_tensor("wza", [CA * WBLK], bf16, kind="ExternalInput")
    wzb_t = nc.dram_tensor("wzb", [CB * WBLK], bf16, kind="ExternalInput")
    out_t = nc.dram_tensor("out", [NOUT, D], bf16, kind="ExternalOutput")

    def pb(t):  # [128,(b,p)] view -> [128, b, p] (p innermost, for reduces)
        return bass.AP(t.tensor, t.offset, [t.ap[0], [K, NB], [1, K]])

    def strip(t, p):  # tap-p strip [128, NB] (stride K)
        return bass.AP(t.tensor, t.offset + p, [t.ap[0], [K, NB]])

    def exp_nb_h(a, b0, nb):  # block-range slice of an exp_nb broadcast
        return bass.AP(a.tensor, a.offset + b0 * a.ap[1][0],
                       [a.ap[0], [a.ap[1][0], nb], [0, K]])

    def exp_k_h(a, nb):       # block-range slice of an exp_k broadcast
        return bass.AP(a.tensor, a.offset, [a.ap[0], [0, nb], [1, K]])

    NB1 = 24                  # block split for vector/gpsimd halves
    NB2 = NB - NB1
    F1 = NB1 * K

    def tt2(out, a, b2, op, b2h=None, ah=None):
        # tensor_tensor split across vector/gpsimd at block boundary NB1.
        if b2h is None:
            b2h = (b2[:, :F1], b2[:, F1:])
        if ah is None:
            ah = (a[:, :F1], a[:, F1:])
        nc.vector.tensor_tensor(out[:, :F1], ah[0], b2h[0], op)
        nc.gpsimd.tensor_tensor(out[:, F1:], ah[1], b2h[1], op)

    with tile.TileContext(nc) as tc:
        with tc.tile_pool(name="alph", bufs=1) as apool, \
             tc.tile_pool(name="outg", bufs=3) as opool, \
             tc.tile_pool(name="big", bufs=1) as bpool, \
             tc.tile_pool(name="psA", bufs=2, space="PSUM") as psA, \
             tc.tile_pool(name="psB", bufs=4, space="PSUM") as psB:

            # ---- input / constant loads (sync = SP ring) ----
            # cst packs dt rows (cols 0:136), identity (136:264),
            # bwsm (264:272) into one DMA.
            cst = apool.tile([128, 272], f32)
            nc.sync.dma_start(cst[:], cst_t.ap())
            vfr = apool.tile([35, 136], f32)
            nc.sync.dma_start(vfr[:], vfr_t.ap())
            dvf = cst
            ident_f = cst[:, 136:264]
            bwsm = cst[:, 264:272]
            # x windows in two chunks: x_all[i, b, d] = x[b*120+i, d]
            x_all = bpool.tile([128, NB, D], bf16)
            for c0, cn in ((0, CA), (CA, CB)):
                nc.sync.dma_start(
                    x_all[:, c0:c0 + cn, :],
                    bass.AP(x_t, c0 * BLK * D,
                            [[D, 128], [BLK * D, cn], [1, D]]))

            # ---- window strips: two PE transposes per tap ----
            # dvf rows 0..34 = dt blocks, rows 64..98 = vf blocks.
            # t1 = BIG - dt fused into the scalar PSUM eviction.
            t1 = apool.tile([128, F], f32)
            vw = apool.tile([128, F], f32)
            cv = apool.tile([128, F], f32)
            lg = apool.tile([128, F], f32)
            for p in range(K):
                ptw = psA.tile([128, 70], f32, tag="win")
                nc.tensor.transpose(ptw[:, 0:NB], dvf[:NB, p:p + 128],
                                    cst[:NB, 136:136 + NB])
                nc.tensor.transpose(ptw[:, NB:70], vfr[:NB, p:p + 128],
                                    cst[:NB, 136:136 + NB])
                nc.scalar.activation(strip(t1, p), ptw[:, 0:NB],
                                     mybir.ActivationFunctionType.Copy,
                                     bias=BIG, scale=-1.0)
                nc.vector.tensor_copy(strip(vw, p), ptw[:, NB:70])
                # cv_p = vw_p * vw_0; lg_p = t1_p * cv_p (under the stream)
                nc.vector.tensor_tensor(strip(cv, p), strip(vw, p),
                                        strip(vw, 0), mybir.AluOpType.mult)
                nc.gpsimd.tensor_tensor(strip(lg, p), strip(t1, p),
                                        strip(cv, p), mybir.AluOpType.mult)

            # ---- alpha stage, fully per chunk ----
            # chunk A: elementwise on vector; chunk B: elementwise on
            # gpsimd (free-axis reduces are vector-only for both). Chunk A
            # finishes first; its skew is emitted immediately below.
            mx = apool.tile([128, NB], f32)
            s = apool.tile([128, NB], f32)
            sa = apool.tile([128, NB], f32)
            r = apool.tile([128, NB], f32)
            ei = apool.tile([128, F], f32)
            e = apool.tile([128, F], f32)
            sc = apool.tile([128, F], f32)
            au = apool.tile([128, F], f32)
            af = apool.tile([128, F], bf16)

            def pb_h(t, c0, cn):
                return bass.AP(t.tensor, t.offset + c0 * K,
                               [t.ap[0], [K, cn], [1, K]])

            def strip_h(t, p, c0, cn):
                return bass.AP(t.tensor, t.offset + p + c0 * K,
                               [t.ap[0], [K, cn]])

            for c0, cn, eng, wt in ((0, CA, nc.vector, wza_t),
                                    (CA, CB, nc.gpsimd, wzb_t)):
                f0, f1 = c0 * K, (c0 + cn) * K
                nc.vector.tensor_reduce(mx[:, c0:c0 + cn], pb_h(lg, c0, cn),
                                        mybir.AxisListType.X,
                                        mybir.AluOpType.max)
                eng.tensor_tensor(ei[:, f0:f1], lg[:, f0:f1],
                                  exp_nb_h(mx[:, :], c0, cn),
                                  mybir.AluOpType.subtract)
                nc.scalar.activation(e[:, f0:f1], ei[:, f0:f1],
                                     mybir.ActivationFunctionType.Exp)
                nc.vector.tensor_reduce(s[:, c0:c0 + cn], pb_h(e, c0, cn),
                                        mybir.AxisListType.X,
                                        mybir.AluOpType.add)
                # scale-invariant blend: au = (e + s*c) * cv (c = bwsm row)
                eng.tensor_tensor(sc[:, f0:f1], exp_nb_h(s[:, :], c0, cn),
                                  exp_k_h(bwsm, cn),
                                  mybir.AluOpType.mult)
                eng.tensor_tensor(au[:, f0:f1], sc[:, f0:f1], e[:, f0:f1],
                                  mybir.AluOpType.add)
                eng.tensor_tensor(au[:, f0:f1], au[:, f0:f1], cv[:, f0:f1],
                                  mybir.AluOpType.mult)
                nc.vector.tensor_reduce(sa[:, c0:c0 + cn], pb_h(au, c0, cn),
                                        mybir.AxisListType.X,
                                        mybir.AluOpType.add)
                nc.vector.tensor_scalar(sa[:, c0:c0 + cn], sa[:, c0:c0 + cn],
                                        1e-8, None, mybir.AluOpType.max)
                nc.vector.reciprocal(r[:, c0:c0 + cn], sa[:, c0:c0 + cn])
                nc.vector.tensor_tensor(r[:, c0:c0 + cn], r[:, c0:c0 + cn],
                                        strip_h(vw, 0, c0, cn),
                                        mybir.AluOpType.mult)
                eng.tensor_tensor(af[:, f0:f1], au[:, f0:f1],
                                  exp_nb_h(r[:, :], c0, cn),
                                  mybir.AluOpType.mult)
                # skewed W write for this chunk (SP ring):
                # W^T[b][m, m+p] = af[m, p, b]
                nc.sync.dma_start(
                    bass.AP(wt, 0, [[KW + 1, BLK], [WBLK, cn], [1, K]]),
                    bass.AP(af.tensor, af.offset + c0 * K,
                            [af.ap[0], [K, cn], [1, K]])[:BLK, :, :])

            # NOTE: the XBAR is one shared unit -- concurrent DMA_TRANSPOSE
            # instructions from the two HWDGE rings corrupt each other.
            # All transposes must serialize on one ring (Act).
            w_all = bpool.tile([128, NB, KW], bf16)
            nc.scalar.dma_start(
                w_all[:, 0:CA, :],
                bass.AP(wza_t, 0, [[KW, CA * KW], [1, KW]]),
                transpose=True)
            nc.scalar.dma_start(
                w_all[:, CA:NB, :],
                bass.AP(wzb_t, 0, [[KW, CB * KW], [1, KW]]),
                transpose=True)

            # ---- per-block banded matmul; paired evict; group stores ----
            for g0, gb in ((0, 8), (8, 8), (16, 8), (24, 8), (32, 3)):
                out_g = opool.tile([128, 8, D], bf16, tag="og")
                for j in range(0, gb, 2):
                    b = g0 + j
                    npair = min(2, gb - j)
                    pt = psB.tile([KW, 2 * D], f32, tag="mm")
                    for q in range(npair):
                        nc.tensor.matmul(pt[:, q * D:(q + 1) * D],
                                         w_all[:, b + q, :],
                                         x_all[:, b + q, :])
                    if (j // 2) % 2 == 1:
                        nc.scalar.copy(out_g[:BLK, j:j + npair, :],
                                       pt[:BLK, :npair * D])
                    else:
                        nc.vector.tensor_copy(out_g[:BLK, j:j + npair, :],
                                              pt[:BLK, :npair * D])
                nc.sync.dma_start(
                    bass.AP(out_t, g0 * BLK * D,
                            [[D, BLK], [BLK * D, gb], [1, D]]),
                    out_g[:BLK, :gb, :])
    nc.compile()
    return nc


def _get_nc():
    if "nc" not in _CACHE:
        _CACHE["nc"] = _build()
    return _CACHE["nc"]


def _make_in_maps(x, delta_times, valid_mask, w, beta):
    w64 = w.astype(np.float64)
    wsm = np.exp(w64 - w64.max())
    wsm /= wsm.sum()
    b = 1.0 / (1.0 + np.exp(-float(beta[0])))
    bwsm = np.tile((b / (1.0 - b) * wsm)[None, :], (128, 1)).astype(np.float32)
    ident = np.eye(128, dtype=np.float32)
    wza = np.zeros(CA * WBLK, np.float32).astype(ml_dtypes.bfloat16)
    wzb = np.zeros(CB * WBLK, np.float32).astype(ml_dtypes.bfloat16)

    in_maps = []
    for i in range(B):
        xp = np.zeros((NPAD, D), np.float32)
        xp[:N] = x[i]
        dtp = np.zeros(NPAD, np.float32)
        dtp[:N] = delta_times[i]
        vfp = np.zeros(NPAD, np.float32)
        vfp[:N] = valid_mask[i].astype(np.float32)
        cst = np.zeros((128, 272), np.float32)
        vfr = np.zeros((35, 136), np.float32)
        for bb in range(NB):
            cst[bb, 0:136] = dtp[bb * BLK:bb * BLK + 136]
            vfr[bb, :] = vfp[bb * BLK:bb * BLK + 136]
        cst[:, 136:264] = ident
        cst[:, 264:272] = bwsm
        in_maps.append({
            "x": xp.astype(ml_dtypes.bfloat16),
            "cst": cst,
            "vfr": vfr,
            "wza": wza,
            "wzb": wzb,
        })
    return in_maps


def _execute(in_maps, trace=False, **kw):
    nc = _get_nc()
    return run_bass_kernel_spmd(nc, in_maps, core_ids=list(range(B)),
                                trace=trace, **kw)


def kernel(x, delta_times, valid_mask, w, beta):
    in_maps = _make_in_maps(x, delta_times, valid_mask, w, beta)
    kr = _execute(in_maps, trace=False)
    outs = [kr.results[i]["out"][:N].astype(np.float32) for i in range(B)]
    return np.stack(outs, axis=0)
